# revision 1
# baseline (speedup 1.0000x reference)
"""Trainium2 Bass kernel for GCBlockP1 (GNN message passing block).

Computation (reference):
    h = tanh(tanh(p1 @ pp_w1 + pp_b1) @ pp_w2 + pp_b2)          [N, D]
    inter = concat(h[idx_i], h[idx_j]) @ pi_w + pi_b            [E, D*B]
    inter = einsum('pcb,pb->pc', inter.reshape(E, D, B), basis) [E, D]
    i1 = tanh(inter @ ii_w + ii_b)                              [E, D]
    out = segment_sum(i1, idx_j, N)                             [N, D]

Strategy (8 NeuronCores, SPMD, zero collectives):
  - Host sorts edges by idx_j, splits into 8 contiguous destination-node
    ranges (~E/8 edges each). Each core only produces node rows in its own
    range, so results concatenate with a trivial host-side overlap-add.
  - Each core recomputes the (small) node MLP for all nodes in bf16 and
    stores h row-major in DRAM.
  - Edges are processed in groups of 16 chunks x 128 edges. Groups are cut
    so all destination nodes of a group fit a 512-node window; short groups
    are padded (pad edges carry jrel=-1 so they never scatter).
  - Per 128-edge chunk: indirect-DMA row gathers of h[idx_i], h[idx_j];
    DMA-transpose; two K=128 bf16 matmuls -> PSUM [e, D*B]; ACT evicts to
    bf16; DVE multiplies by broadcast basis and group-of-8-reduces (the
    einsum); DMA-transpose; ii matmul; ACT tanh; scatter via an is_equal
    segment-indicator matmul accumulating [d, 512] in PSUM across the
    group's 16 chunks; PSUM -> DRAM staging per group.
  - Host adds staging slabs into the output at each group's base node.
"""

import numpy as np
import ml_dtypes

import concourse.bass as bass
import concourse.bacc as bacc
import concourse.mybir as mybir
import concourse.tile as tile
from concourse.bass_utils import run_bass_kernel_spmd

BF16 = ml_dtypes.bfloat16

NCORES = 8
D = 128
NB = 8
CHUNK = 128          # edges per chunk (one SBUF partition set)
CPG = 16             # chunks per group
GROUP = CHUNK * CPG  # 2048 edge slots per group
WIN = 512            # destination-node window per group
PPT = 512            # nodes per pp-phase tile


# ---------------------------------------------------------------------------
# Host-side planning
# ---------------------------------------------------------------------------

def _plan(idx_i, idx_j, basis, n_nodes, ncores):
    """Sort edges by destination, split across cores at node boundaries,
    cut into (<=GROUP edges, <=WIN node-span) groups, pack device arrays."""
    E = idx_i.shape[0]
    order = np.argsort(idx_j, kind="stable")
    ji = idx_j[order]

    starts = [0]
    for c in range(1, ncores):
        pos = c * E // ncores
        pos = int(np.searchsorted(ji, ji[pos], side="left"))
        starts.append(pos)
    starts.append(E)

    per_core_groups = []
    for c in range(ncores):
        lo, hi = starts[c], starts[c + 1]
        jc = ji[lo:hi]
        oc = order[lo:hi]
        groups = []
        p = 0
        while p < len(jc):
            base = int(jc[p])
            pend = min(p + GROUP, len(jc))
            pend = min(pend, int(np.searchsorted(jc, base + WIN, side="left")))
            groups.append((base, oc[p:pend]))
            p = pend
        per_core_groups.append(groups)

    G = max(len(g) for g in per_core_groups)

    cores = []
    for c in range(ncores):
        groups = per_core_groups[c]
        gi = np.zeros((G, GROUP), np.int32)
        gj = np.zeros((G, GROUP), np.int32)
        jr = np.full((G, GROUP), -1, np.float32)
        bs = np.zeros((G, GROUP, NB), np.float32)
        bases = np.zeros(G, np.int64)
        for g, (base, sel) in enumerate(groups):
            n = len(sel)
            gi[g, :n] = idx_i[sel]
            gj[g, :n] = idx_j[sel]
            jr[g, :n] = (idx_j[sel] - base).astype(np.float32)
            bs[g, :n] = basis[sel]
            bases[g] = base
        # device layout: slot (q, p) = edge q*CHUNK+p  ->  [G, p(128), q(16)]
        # merged gather offsets: [G, p(128), q(16), 2] with (idx_i, idx_j)
        gij = np.stack([gi.reshape(G, CPG, CHUNK).transpose(0, 2, 1),
                        gj.reshape(G, CPG, CHUNK).transpose(0, 2, 1)], axis=-1)
        cores.append(dict(
            gij=np.ascontiguousarray(gij),
            jr=np.ascontiguousarray(jr.reshape(G, CPG, CHUNK).transpose(0, 2, 1)),
            bs=np.ascontiguousarray(
                bs.reshape(G, CPG, CHUNK, NB).transpose(0, 2, 1, 3)).astype(BF16),
            bases=bases,
            ngroups=len(groups),
        ))
    return cores, G


# ---------------------------------------------------------------------------
# Device program
# ---------------------------------------------------------------------------

def _bcast_mid(ap, count):
    """[P, k] AP -> [P, count, k] AP with a stride-0 middle dim."""
    return bass.AP(ap.tensor, ap.offset, [ap.ap[0], [0, count], ap.ap[1]])


def _build(npad, G, nz_pib, nz_iib, repeat=1):
    nc = bacc.Bacc("TRN2", num_swdge_queues=2)
    f32, bf16 = mybir.dt.float32, mybir.dt.bfloat16
    i32, i16 = mybir.dt.int32, mybir.dt.int16

    p1b = nc.dram_tensor("p1b", [npad, D], bf16, kind="ExternalInput")
    w1 = nc.dram_tensor("w1", [D, D], bf16, kind="ExternalInput")
    w2 = nc.dram_tensor("w2", [D, D], bf16, kind="ExternalInput")
    b1 = nc.dram_tensor("b1", [D, 1], f32, kind="ExternalInput")
    b2 = nc.dram_tensor("b2", [D, 1], f32, kind="ExternalInput")
    piwi = nc.dram_tensor("piwi", [D, D * NB], bf16, kind="ExternalInput")
    piwj = nc.dram_tensor("piwj", [D, D * NB], bf16, kind="ExternalInput")
    iiw = nc.dram_tensor("iiw", [D, D], bf16, kind="ExternalInput")
    gij = nc.dram_tensor("gij", [G, CHUNK, CPG, 2], i32, kind="ExternalInput")
    jr = nc.dram_tensor("jr", [G, CHUNK, CPG], f32, kind="ExternalInput")
    bas = nc.dram_tensor("bas", [G, CHUNK, CPG, NB], bf16, kind="ExternalInput")
    if nz_pib:
        pibr = nc.dram_tensor("pibr", [CHUNK, D * NB], bf16, kind="ExternalInput")
    if nz_iib:
        iibr = nc.dram_tensor("iibr", [CHUNK, D], bf16, kind="ExternalInput")

    staging = nc.dram_tensor("staging", [G, D, WIN], f32, kind="ExternalOutput")
    h_dram = nc.dram_tensor("h_dram", [npad, D], bf16)

    npp = npad // PPT

    with tile.TileContext(nc) as tc:
        with tc.tile_pool(name="const", bufs=1) as cpool:
            w1_t = cpool.tile([D, D], bf16)
            w2_t = cpool.tile([D, D], bf16)
            b1_t = cpool.tile([D, 1], f32)
            b2_t = cpool.tile([D, 1], f32)
            piwi_t = cpool.tile([D, D * NB], bf16)
            piwj_t = cpool.tile([D, D * NB], bf16)
            iiw_t = cpool.tile([D, D], bf16)
            iota_t = cpool.tile([CHUNK, WIN], i16)
            nc.sync.dma_start(out=w1_t[:], in_=w1[:])
            nc.sync.dma_start(out=w2_t[:], in_=w2[:])
            nc.sync.dma_start(out=b1_t[:], in_=b1[:])
            nc.sync.dma_start(out=b2_t[:], in_=b2[:])
            nc.sync.dma_start(out=piwi_t[:], in_=piwi[:])
            nc.sync.dma_start(out=piwj_t[:], in_=piwj[:])
            nc.sync.dma_start(out=iiw_t[:], in_=iiw[:])
            nc.gpsimd.iota(iota_t[:], [[1, WIN]], channel_multiplier=0)
            if nz_pib:
                pibr_t = cpool.tile([CHUNK, D * NB], bf16)
                nc.sync.dma_start(out=pibr_t[:], in_=pibr[:])
            if nz_iib:
                iibr_t = cpool.tile([CHUNK, D], bf16)
                nc.sync.dma_start(out=iibr_t[:], in_=iibr[:])

            for _rep in range(repeat):
                _build_phases(nc, tc, locals())
    nc.compile()
    return nc


def _build_phases(nc, tc, env):
    (f32, bf16, i32, i16) = (mybir.dt.float32, mybir.dt.bfloat16,
                             mybir.dt.int32, mybir.dt.int16)
    w1_t = env["w1_t"]; w2_t = env["w2_t"]; b1_t = env["b1_t"]
    b2_t = env["b2_t"]; piwi_t = env["piwi_t"]; piwj_t = env["piwj_t"]
    iiw_t = env["iiw_t"]; iota_t = env["iota_t"]
    p1b = env["p1b"]; h_dram = env["h_dram"]; npp = env["npp"]
    gij = env["gij"]; jr = env["jr"]; bas = env["bas"]
    staging = env["staging"]; G = env["G"]
    nz_pib = env["nz_pib"]; nz_iib = env["nz_iib"]
    pibr_t = env.get("pibr_t"); iibr_t = env.get("iibr_t")
    if True:
        if True:
            # ---- phase 1: node MLP, h = tanh(tanh(p1@w1+b1)@w2+b2) ----
            # h-row writes for tile t are deferred to iteration t+1 so the
            # in-order SP stream never stalls on tile t's tanh.
            with tc.tile_pool(name="pp", bufs=4) as pp, \
                 tc.tile_pool(name="ppp", bufs=2, space="PSUM") as ppp:
                def pp_writes(h2, t):
                    for qq in range(PPT // D):
                        hr = pp.tile([D, D], bf16, name="hr", tag="hr")
                        nc.sync.dma_start_transpose(
                            hr[:], h2[:, qq * D:(qq + 1) * D])
                        nc.sync.dma_start(
                            out=h_dram[t * PPT + qq * D:t * PPT + (qq + 1) * D, :],
                            in_=hr[:])

                prev = None
                for t in range(npp):
                    p1T = pp.tile([D, PPT], bf16)
                    nc.sync.dma_start_transpose(
                        p1T[:], p1b[t * PPT:(t + 1) * PPT, :])
                    ps1 = ppp.tile([D, PPT], f32)
                    nc.tensor.matmul(out=ps1[:], lhsT=w1_t[:], rhs=p1T[:],
                                     start=True, stop=True)
                    h1 = pp.tile([D, PPT], bf16)
                    nc.scalar.activation(h1[:], ps1[:],
                                         mybir.ActivationFunctionType.Tanh,
                                         bias=b1_t[:, :1])
                    ps2 = ppp.tile([D, PPT], f32)
                    nc.tensor.matmul(out=ps2[:], lhsT=w2_t[:], rhs=h1[:],
                                     start=True, stop=True)
                    h2 = pp.tile([D, PPT], bf16)
                    nc.scalar.activation(h2[:], ps2[:],
                                         mybir.ActivationFunctionType.Tanh,
                                         bias=b2_t[:, :1])
                    if prev is not None:
                        pp_writes(*prev)
                    prev = (h2, t)
                pp_writes(*prev)

            # ---- phase 2: edges ----
            # Flat chunk pipeline, software-pipelined so every consumer runs
            # well after its producer's latency: the ii matmul for chunk k is
            # emitted at k+DELAY_II, the scatter matmul at k+DELAY_SC (one
            # full group), so the in-order PE stream never stalls.
            DELAY_EIN = 3
            DELAY_S = 3
            DELAY_IRT = 5
            DELAY_II = 10
            DELAY_SC = CPG
            K = G * CPG
            with tc.tile_pool(name="eg", bufs=3) as eg, \
                 tc.tile_pool(name="ew", bufs=12) as ew, \
                 tc.tile_pool(name="late", bufs=DELAY_SC + 14) as late, \
                 tc.tile_pool(name="psI", bufs=2, space="PSUM") as psIp, \
                 tc.tile_pool(name="psJ", bufs=2, space="PSUM") as psJp, \
                 tc.tile_pool(name="psA", bufs=2, space="PSUM") as psAp:
                groups = {}   # g -> (git, jrt, bat)
                gaths = {}    # k -> (tile, col offset)
                sbIs = {}     # k -> evicted inter (awaiting einsum)
                ireds = {}    # k -> ired tile (awaiting irT transpose)
                irTs = {}     # k -> transposed ired (awaiting ii matmul)
                i1s = {}      # k -> i1 tile (awaiting scatter)
                Ss = {}       # k -> S tile (awaiting scatter)
                psAs = {}     # g -> psum accumulator

                def load_group(g):
                    git = eg.tile([CHUNK, CPG * 2], i32)
                    jrt = eg.tile([CHUNK, CPG], f32)
                    bat = eg.tile([CHUNK, CPG * NB], bf16)
                    nc.sync.dma_start(
                        out=git[:], in_=gij[g].rearrange("p q t -> p (q t)"))
                    nc.sync.dma_start(out=jrt[:], in_=jr[g])
                    nc.sync.dma_start(
                        out=bat[:], in_=bas[g].rearrange("p q b -> p (q b)"))
                    groups[g] = (git, jrt, bat)

                def issue_gather_pair(k):
                    for kk in (k, k + 1):
                        g, q = divmod(kk, CPG)
                        git = groups[g][0]
                        gath = ew.tile([CHUNK, 2 * D], bf16, name="gath",
                                       tag="gath", bufs=20)
                        g1 = nc.gpsimd.indirect_dma_start(
                            out=gath[:, 0:D], out_offset=None, in_=h_dram[:],
                            in_offset=bass.IndirectOffsetOnAxis(
                                ap=git[:, 2 * q:2 * q + 1], axis=0))
                        g2 = nc.gpsimd.indirect_dma_start(
                            out=gath[:, D:2 * D], out_offset=None, in_=h_dram[:],
                            in_offset=bass.IndirectOffsetOnAxis(
                                ap=git[:, 2 * q + 1:2 * q + 2], axis=0))
                        g2.ins.queue = "qPoolDynamic1"
                        gaths[kk] = (gath, 0)

                hTs = {}

                def issue_transposes(k):
                    gath, off = gaths.pop(k)
                    hTi = ew.tile([D, CHUNK], bf16, name="hTi", tag="hTi")
                    hTj = ew.tile([D, CHUNK], bf16, name="hTj", tag="hTj")
                    nc.sync.dma_start_transpose(hTi[:], gath[:, off:off + D])
                    nc.sync.dma_start_transpose(
                        hTj[:], gath[:, off + D:off + 2 * D])
                    hTs[k] = (hTi, hTj)

                def compute_chunk(k):
                    g, q = divmod(k, CPG)
                    git, jrt, bat = groups[g]
                    hTi, hTj = hTs.pop(k)

                    psI = psIp.tile([CHUNK, D * NB], f32)
                    half = D * NB // 2
                    nc.tensor.matmul(out=psI[:, 0:half], lhsT=hTi[:],
                                     rhs=piwi_t[:, 0:half],
                                     start=True, stop=False)
                    nc.tensor.matmul(out=psI[:, half:], lhsT=hTi[:],
                                     rhs=piwi_t[:, half:],
                                     start=True, stop=False)
                    nc.tensor.matmul(out=psI[:, 0:half], lhsT=hTj[:],
                                     rhs=piwj_t[:, 0:half],
                                     start=False, stop=True)
                    nc.tensor.matmul(out=psI[:, half:], lhsT=hTj[:],
                                     rhs=piwj_t[:, half:],
                                     start=False, stop=True)

                    sbI = late.tile([CHUNK, D * NB], bf16, tag="sbI", bufs=8)
                    nc.scalar.activation(sbI[:], psI[:],
                                         mybir.ActivationFunctionType.Copy)
                    if nz_pib:
                        nc.vector.tensor_tensor(
                            out=sbI[:], in0=sbI[:], in1=pibr_t[:],
                            op=mybir.AluOpType.add)
                    sbIs[k] = sbI

                def einsum_chunk(k):
                    g, q = divmod(k, CPG)
                    git, jrt, bat = groups[g]
                    sbI = sbIs.pop(k)
                    prod = ew.tile([CHUNK, D * NB], bf16)
                    nc.vector.tensor_tensor(
                        out=prod[:], in0=sbI[:],
                        in1=_bcast_mid(bat[:, q * NB:(q + 1) * NB], D),
                        op=mybir.AluOpType.mult)
                    # reduce groups of NB=8 via a 3-level pairwise tree (TT
                    # adds run 2x_1P; tensor_reduce would run 1x).
                    r1 = ew.tile([CHUNK, D * 4], bf16)
                    p3 = prod[:].rearrange("p (c b) -> p c b", b=NB)
                    nc.vector.tensor_tensor(
                        out=r1[:].rearrange("p (c b) -> p c b", b=4),
                        in0=p3[:, :, 0:4], in1=p3[:, :, 4:8],
                        op=mybir.AluOpType.add)
                    r2 = ew.tile([CHUNK, D * 2], bf16)
                    r1v = r1[:].rearrange("p (c b) -> p c b", b=4)
                    nc.vector.tensor_tensor(
                        out=r2[:].rearrange("p (c b) -> p c b", b=2),
                        in0=r1v[:, :, 0:2], in1=r1v[:, :, 2:4],
                        op=mybir.AluOpType.add)
                    ired = late.tile([CHUNK, D], bf16, tag="ired", bufs=8)
                    r2v = r2[:].rearrange("p (c b) -> p c b", b=2)
                    nc.vector.tensor_tensor(
                        out=ired[:], in0=r2v[:, :, 0], in1=r2v[:, :, 1],
                        op=mybir.AluOpType.add)
                    ireds[k] = ired

                def sbuild_chunk(k):
                    g, q = divmod(k, CPG)
                    jrt = groups[g][1]
                    S = late.tile([CHUNK, WIN], bf16, tag="S")
                    nc.vector.tensor_scalar(
                        out=S[:], in0=iota_t[:], scalar1=jrt[:, q:q + 1],
                        scalar2=None, op0=mybir.AluOpType.is_equal)
                    Ss[k] = S

                def irt_chunk(k):
                    ired = ireds.pop(k)
                    irT = late.tile([D, CHUNK], bf16, tag="irT", bufs=12)
                    nc.sync.dma_start_transpose(irT[:], ired[:])
                    irTs[k] = irT

                def ii_chunk(k):
                    irT = irTs.pop(k)
                    psJ = psJp.tile([CHUNK, D], f32)
                    nc.tensor.matmul(out=psJ[:], lhsT=irT[:], rhs=iiw_t[:],
                                     start=True, stop=True)
                    i1 = late.tile([CHUNK, D], bf16, tag="i1")
                    if nz_iib:
                        tmp = late.tile([CHUNK, D], bf16, tag="i1tmp")
                        nc.vector.tensor_tensor(
                            out=tmp[:], in0=psJ[:], in1=iibr_t[:],
                            op=mybir.AluOpType.add)
                        nc.scalar.activation(
                            i1[:], tmp[:], mybir.ActivationFunctionType.Tanh)
                    else:
                        nc.scalar.activation(
                            i1[:], psJ[:], mybir.ActivationFunctionType.Tanh)
                    i1s[k] = i1

                def scatter_chunk(k):
                    g, q = divmod(k, CPG)
                    if q == 0:
                        psAs[g] = psAp.tile([D, WIN], f32, name="psA", tag="psA")
                    nc.tensor.matmul(out=psAs[g][:], lhsT=i1s.pop(k)[:],
                                     rhs=Ss.pop(k)[:],
                                     start=(q == 0), stop=(q == CPG - 1))
                    if q == CPG - 1:
                        psA = psAs.pop(g)
                        acc_sb = eg.tile([D, WIN], f32)
                        nc.scalar.activation(acc_sb[:], psA[:],
                                             mybir.ActivationFunctionType.Copy)
                        nc.sync.dma_start(out=staging[g], in_=acc_sb[:])

                GA = 10
                TA = 2
                load_group(0)
                for j in range(0, min(GA, K), 2):
                    issue_gather_pair(j)
                for j in range(0, min(TA, K)):
                    issue_transposes(j)
                for k in range(K + DELAY_SC):
                    ka = k + GA
                    if ka < K and ka % CPG == 0:
                        load_group(ka // CPG)
                    if ka < K and ka % 2 == 0:
                        issue_gather_pair(ka)
                    if k + TA < K:
                        issue_transposes(k + TA)
                    if k < K:
                        compute_chunk(k)
                    if 0 <= k - DELAY_EIN < K:
                        einsum_chunk(k - DELAY_EIN)
                    if 0 <= k - DELAY_S < K:
                        sbuild_chunk(k - DELAY_S)
                    if 0 <= k - DELAY_IRT < K:
                        irt_chunk(k - DELAY_IRT)
                    if 0 <= k - DELAY_II < K:
                        ii_chunk(k - DELAY_II)
                    if 0 <= k - DELAY_SC < K:
                        scatter_chunk(k - DELAY_SC)


# ---------------------------------------------------------------------------
# Entry point
# ---------------------------------------------------------------------------

def _prep_inputs(p1, idx_i, idx_j, basis, pp_w1, pp_b1, pp_w2, pp_b2,
                 pi_w, pi_b, ii_w, ii_b, ncores):
    n_nodes = p1.shape[0]
    npad = ((n_nodes + PPT - 1) // PPT) * PPT
    p1b = np.zeros((npad, D), BF16)
    p1b[:n_nodes] = p1.astype(BF16)

    cores, G = _plan(np.asarray(idx_i), np.asarray(idx_j), np.asarray(basis),
                     n_nodes, ncores)

    nz_pib = bool(np.any(pi_b != 0))
    nz_iib = bool(np.any(ii_b != 0))

    common = dict(
        p1b=p1b,
        w1=pp_w1.astype(BF16), w2=pp_w2.astype(BF16),
        b1=pp_b1.astype(np.float32).reshape(D, 1),
        b2=pp_b2.astype(np.float32).reshape(D, 1),
        piwi=pi_w[:D].astype(BF16), piwj=pi_w[D:].astype(BF16),
        iiw=ii_w.astype(BF16),
    )
    if nz_pib:
        common["pibr"] = np.tile(pi_b.astype(BF16)[None, :], (CHUNK, 1))
    if nz_iib:
        common["iibr"] = np.tile(ii_b.astype(BF16)[None, :], (CHUNK, 1))

    in_maps = []
    for c in range(ncores):
        m = dict(common)
        m["gij"] = cores[c]["gij"]
        m["jr"] = cores[c]["jr"]
        m["bas"] = cores[c]["bs"]
        in_maps.append(m)
    return in_maps, cores, G, npad, n_nodes, nz_pib, nz_iib


def _assemble(results, cores, n_nodes):
    out = np.zeros((n_nodes, D), np.float32)
    for c, core in enumerate(cores):
        st = results[c]["staging"]
        for g in range(core["ngroups"]):
            base = int(core["bases"][g])
            w = min(WIN, n_nodes - base)
            out[base:base + w] += st[g, :, :w].T
    return out


LAST_RESULTS = None


def kernel(p1, idx_i, idx_j, basis, pp_w1, pp_b1, pp_w2, pp_b2,
           pi_w, pi_b, ii_w, ii_b):
    global LAST_RESULTS
    in_maps, cores, G, npad, n_nodes, nz_pib, nz_iib = _prep_inputs(
        p1, idx_i, idx_j, basis, pp_w1, pp_b1, pp_w2, pp_b2,
        pi_w, pi_b, ii_w, ii_b, NCORES)
    nc = _build(npad, G, nz_pib, nz_iib)
    res = run_bass_kernel_spmd(nc, in_maps, core_ids=list(range(NCORES)))
    LAST_RESULTS = res
    return _assemble(res.results, cores, n_nodes)



# revision 40
# speedup vs baseline: 7.1511x; 7.1511x over previous
"""Trainium2 Bass kernel for GCBlockP1 (GNN message passing block).

Computation (reference):
    h = tanh(tanh(p1 @ pp_w1 + pp_b1) @ pp_w2 + pp_b2)          [N, D]
    inter = concat(h[idx_i], h[idx_j]) @ pi_w + pi_b            [E, D*B]
    inter = einsum('pcb,pb->pc', inter.reshape(E, D, B), basis) [E, D]
    i1 = tanh(inter @ ii_w + ii_b)                              [E, D]
    out = segment_sum(i1, idx_j, N)                             [N, D]

Strategy (8 NeuronCores, SPMD, zero collectives):
  - Host sorts edges by idx_j, splits into 8 contiguous destination-node
    ranges (~E/8 edges each).  Each core only produces rows in its own
    range; host does a trivial overlap-add of per-group staging slabs.
  - ii_w is folded into pi_w host-side (einsum over basis commutes with
    the c-contraction):  piw_eff[x,(y,b)] = sum_c piw[x,(c,b)] iiw[c,y].
    This removes the ii matmul and the ired transpose from the device
    program entirely; i1 = tanh(einsum(X @ piw_eff, basis)).
  - Each core computes the node MLP only for the ~32k distinct nodes its
    edges reference (compact tables, int16-indexable): a table of unique
    i-endpoints plus the core's contiguous j-range.  p1 columns for both
    are uploaded pre-transposed, so phase 1 needs no input transposes;
    row-major h output is produced with PE transposes (identity matmul).
  - Edge phase: per group of 2048 edges, two batched transposing SWDGE
    gathers (dma_gather transpose=True) deliver hT_i / hT_j columns
    directly (994ns fixed descgen cost amortized over the group).  Per
    128-edge chunk: 4 matmuls accumulate psI[e, D*B]; ACT evicts to
    bf16; DVE multiplies by broadcast basis and starts the group-of-8
    reduction; Pool finishes it; ACT tanh -> i1; scatter via an
    is_equal segment-indicator matmul accumulating [d, 512] in PSUM
    across the group's 16 chunks; PSUM -> DRAM staging per group.
"""

import numpy as np
import ml_dtypes

import concourse.bass as bass
import concourse.bacc as bacc
import concourse.mybir as mybir
import concourse.tile as tile
from concourse.bass_utils import run_bass_kernel_spmd

BF16 = ml_dtypes.bfloat16

NCORES = 8
D = 128
NB = 8
CHUNK = 128          # edges per chunk (one SBUF partition set)
CPG = 16             # chunks per group
GROUP = CHUNK * CPG  # 2048 edge slots per group
WIN = 256            # destination-node window per group
PPT = 512            # nodes per pp-phase tile


# ---------------------------------------------------------------------------
# Host-side planning
# ---------------------------------------------------------------------------

def _pad_to(x, m):
    return ((x + m - 1) // m) * m


def _wrap16(arr):
    """[GROUP] int array -> [128, GROUP//16] i16 wrapped+replicated layout:
    idx n lives at partition n%16, column n//16; replicated to all 8
    16-partition core groups."""
    w = arr.reshape(GROUP // 16, 16).T.astype(np.int16)   # [16, GROUP//16]
    return np.tile(w, (8, 1))                             # [128, GROUP//16]


def _plan(idx_i, idx_j, basis, n_nodes, ncores):
    """Sort edges by destination, split across cores at node boundaries,
    cut into (<=GROUP edges, <=WIN node-span) groups, build compact node
    tables and pack device arrays."""
    E = idx_i.shape[0]
    order = np.argsort(idx_j, kind="stable")
    ji = idx_j[order]

    starts = [0]
    for c in range(1, ncores):
        pos = c * E // ncores
        pos = int(np.searchsorted(ji, ji[pos], side="left"))
        starts.append(pos)
    starts.append(E)

    cores = []
    for c in range(ncores):
        lo, hi = starts[c], starts[c + 1]
        oc = order[lo:hi]
        jc = ji[lo:hi]
        ic = idx_i[oc]

        # compact i table (unique endpoints, sorted) and j range
        uniq_i = np.unique(ic)
        irel = np.searchsorted(uniq_i, ic).astype(np.int64)
        jmin = int(jc[0])
        jspan = int(jc[-1]) + 1 - jmin
        ni_pad = _pad_to(len(uniq_i), PPT)
        nj_pad = _pad_to(jspan, PPT)
        assert ni_pad <= 32767 and nj_pad <= 32767

        groups = []
        p = 0
        while p < len(jc):
            base = int(jc[p])
            pend = min(p + GROUP, len(jc))
            pend = min(pend, int(np.searchsorted(jc, base + WIN, side="left")))
            groups.append((base, p, pend))
            p = pend
        cores.append(dict(uniq_i=uniq_i, irel=irel, jmin=jmin,
                          ni_pad=ni_pad, nj_pad=nj_pad,
                          oc=oc, jc=jc, groups=groups))

    G = max(len(core["groups"]) for core in cores)
    ni_pad = max(core["ni_pad"] for core in cores)
    nj_pad = max(core["nj_pad"] for core in cores)

    for core in cores:
        core["ni_pad"] = ni_pad
        core["nj_pad"] = nj_pad
        groups = core["groups"]
        jc, oc, irel, jmin = core["jc"], core["oc"], core["irel"], core["jmin"]
        # one packed byte tensor per group: wi | wj | jr | bas  (one DMA)
        WIB = GROUP // 16 * 2                 # 256B of i16
        JRB = CPG * 4                         # 64B of f32
        BAB = CPG * NB * 2                    # 256B of bf16
        GLB = 2 * WIB + JRB + BAB             # 832B per partition
        gl = np.zeros((G, 128, GLB), np.uint8)
        bases = np.zeros(G, np.int64)
        for g, (base, p0, p1_) in enumerate(groups):
            n = p1_ - p0
            ai = np.zeros(GROUP, np.int64)
            aj = np.zeros(GROUP, np.int64)
            ai[:n] = irel[p0:p1_]
            aj[:n] = jc[p0:p1_] - jmin
            gl[g, :, 0:WIB] = _wrap16(ai).view(np.uint8)
            gl[g, :, WIB:2 * WIB] = _wrap16(aj).view(np.uint8)
            jrel = np.full(GROUP, -1.0, np.float32)
            jrel[:n] = (jc[p0:p1_] - base).astype(np.float32)
            # slot s = q*CHUNK + p -> tile [p, q]
            gl[g, :, 2 * WIB:2 * WIB + JRB] = np.ascontiguousarray(
                jrel.reshape(CPG, CHUNK).T).view(np.uint8)
            bg = np.zeros((GROUP, NB), np.float32)
            bg[:n] = basis[oc[p0:p1_]]
            bs = np.ascontiguousarray(
                bg.reshape(CPG, CHUNK, NB).transpose(1, 0, 2).reshape(
                    CHUNK, CPG * NB).astype(BF16))
            gl[g, :, 2 * WIB + JRB:] = bs.view(np.uint8)
            bases[g] = base
        core.update(gl=gl, bases=bases, ngroups=len(groups))
    return cores, G, ni_pad, nj_pad


# ---------------------------------------------------------------------------
# Device program
# ---------------------------------------------------------------------------

def _bcast_mid(ap, count):
    """[P, k] AP -> [P, count, k] AP with a stride-0 middle dim."""
    return bass.AP(ap.tensor, ap.offset, [ap.ap[0], [0, count], ap.ap[1]])


class _Cfg:
    def __init__(self, G, ni_pad, nj_pad, nz_pib, nz_iib):
        self.G, self.ni_pad, self.nj_pad = G, ni_pad, nj_pad
        self.nz_pib, self.nz_iib = nz_pib, nz_iib


def _build(cfg, repeat=1):
    nc = bacc.Bacc("TRN2", num_swdge_queues=2)
    f32, bf16 = mybir.dt.float32, mybir.dt.bfloat16
    i16 = mybir.dt.int16
    G, ni_pad, nj_pad = cfg.G, cfg.ni_pad, cfg.nj_pad
    ntab = ni_pad + nj_pad

    tabT = nc.dram_tensor("tabT", [D, ntab], bf16, kind="ExternalInput")
    w1 = nc.dram_tensor("w1", [D, D], bf16, kind="ExternalInput")
    w2 = nc.dram_tensor("w2", [D, D], bf16, kind="ExternalInput")
    b1 = nc.dram_tensor("b1", [D, 1], f32, kind="ExternalInput")
    b2 = nc.dram_tensor("b2", [D, 1], f32, kind="ExternalInput")
    piwi = nc.dram_tensor("piwi", [D, D * NB], bf16, kind="ExternalInput")
    piwj = nc.dram_tensor("piwj", [D, D * NB], bf16, kind="ExternalInput")
    GLB = GROUP // 16 * 2 * 2 + CPG * 4 + CPG * NB * 2
    gld = nc.dram_tensor("gld", [G, 128, GLB], mybir.dt.uint8,
                         kind="ExternalInput")
    if cfg.nz_pib:
        pibr = nc.dram_tensor("pibr", [CHUNK, D * NB], bf16,
                              kind="ExternalInput")
    if cfg.nz_iib:
        iibr = nc.dram_tensor("iibr", [CHUNK, D], bf16, kind="ExternalInput")

    staging = nc.dram_tensor("staging", [G, D, WIN], f32,
                             kind="ExternalOutput")
    h_dram = nc.dram_tensor("h_dram", [ntab, D], bf16)

    with tile.TileContext(nc) as tc:
        with tc.tile_pool(name="const", bufs=1) as cpool:
            w1_t = cpool.tile([D, D], bf16)
            w2_t = cpool.tile([D, D], bf16)
            b1_t = cpool.tile([D, 1], f32)
            b2_t = cpool.tile([D, 1], f32)
            piwi_t = cpool.tile([D, D * NB], bf16)
            piwj_t = cpool.tile([D, D * NB], bf16)
            iota_t = cpool.tile([CHUNK, WIN], i16)
            iotaF = cpool.tile([D, D], i16)
            iotaP = cpool.tile([D, 1], f32)
            ident = cpool.tile([D, D], bf16)
            nc.sync.dma_start(out=w1_t[:], in_=w1[:])
            nc.sync.dma_start(out=w2_t[:], in_=w2[:])
            nc.sync.dma_start(out=b1_t[:], in_=b1[:])
            nc.sync.dma_start(out=b2_t[:], in_=b2[:])
            nc.sync.dma_start(out=piwi_t[:], in_=piwi[:])
            nc.sync.dma_start(out=piwj_t[:], in_=piwj[:])
            nc.gpsimd.iota(iota_t[:], [[1, WIN]], channel_multiplier=0)
            nc.gpsimd.iota(iotaF[:], [[1, D]], channel_multiplier=0)
            nc.gpsimd.iota(iotaP[:], [[1, 1]], channel_multiplier=1,
                           allow_small_or_imprecise_dtypes=True)
            nc.vector.tensor_scalar(
                out=ident[:], in0=iotaF[:], scalar1=iotaP[:, 0:1],
                scalar2=None, op0=mybir.AluOpType.is_equal)
            if cfg.nz_pib:
                pibr_t = cpool.tile([CHUNK, D * NB], bf16)
                nc.sync.dma_start(out=pibr_t[:], in_=pibr[:])
            if cfg.nz_iib:
                iibr_t = cpool.tile([CHUNK, D], bf16)
                nc.sync.dma_start(out=iibr_t[:], in_=iibr[:])

            env = dict(locals())
            for _rep in range(repeat):
                _phase1(nc, tc, env)
                _phase2(nc, tc, env)
    nc.compile()
    return nc


def _phase1(nc, tc, env):
    """h = tanh(tanh(tabT.T @ w1 + b1) @ w2 + b2) written row-major to
    h_dram.  Input arrives pre-transposed; output rows via PE transpose."""
    f32, bf16 = mybir.dt.float32, mybir.dt.bfloat16
    tabT, h_dram = env["tabT"], env["h_dram"]
    w1_t, w2_t = env["w1_t"], env["w2_t"]
    b1_t, b2_t = env["b1_t"], env["b2_t"]
    ident = env["ident"]
    ntab = env["cfg"].ni_pad + env["cfg"].nj_pad
    npp = ntab // PPT

    with tc.tile_pool(name="pp", bufs=4) as pp, \
         tc.tile_pool(name="ppe", bufs=3) as ppe, \
         tc.tile_pool(name="ppp", bufs=2, space="PSUM") as ppp, \
         tc.tile_pool(name="ppt", bufs=4, space="PSUM") as ppt:

        def pp_writes(h2, t):
            # PE-transpose the tile back to row-major into ONE PSUM tile and
            # flush straight to DRAM with ONE DMA (HWDGE cost is
            # per-instruction, not per-byte; DMA engines can read PSUM)
            psT4 = ppt.tile([D, PPT], bf16, name="psT4", tag="psT4")
            for qq in range(PPT // D):
                nc.tensor.transpose(psT4[:, qq * D:(qq + 1) * D],
                                    h2[:, qq * D:(qq + 1) * D], ident[:])
            hrw = ppe.tile([D, PPT], bf16, name="hrw", tag="hrw")
            nc.vector.tensor_copy(hrw[:], psT4[:])
            nc.sync.dma_start(
                out=h_dram[t * PPT:(t + 1) * PPT, :].rearrange(
                    "(q p) d -> p q d", p=D),
                in_=hrw[:].rearrange("p (q d) -> p q d", d=D))

        prev = None
        xT2 = None
        for t in range(npp):
            if t % 2 == 0:
                width = min(2 * PPT, (npp - t) * PPT)
                xT2 = pp.tile([D, 2 * PPT], bf16, name="xT2", tag="xT2")
                nc.sync.dma_start(out=xT2[:, 0:width],
                                  in_=tabT[:, t * PPT:t * PPT + width])
            xT = xT2[:, (t % 2) * PPT:(t % 2 + 1) * PPT]
            ps1 = ppp.tile([D, PPT], f32)
            nc.tensor.matmul(out=ps1[:], lhsT=w1_t[:], rhs=xT,
                             start=True, stop=True)
            h1 = pp.tile([D, PPT], bf16)
            nc.scalar.activation(h1[:], ps1[:],
                                 mybir.ActivationFunctionType.Tanh,
                                 bias=b1_t[:, :1])
            ps2 = ppp.tile([D, PPT], f32)
            nc.tensor.matmul(out=ps2[:], lhsT=w2_t[:], rhs=h1[:],
                             start=True, stop=True)
            h2 = pp.tile([D, PPT], bf16)
            nc.scalar.activation(h2[:], ps2[:],
                                 mybir.ActivationFunctionType.Tanh,
                                 bias=b2_t[:, :1])
            if prev is not None:
                pp_writes(*prev)
            prev = (h2, t)
        pp_writes(*prev)


def _phase2(nc, tc, env):
    """Edge phase: batched transposing gathers + chunk pipeline."""
    f32, bf16 = mybir.dt.float32, mybir.dt.bfloat16
    i16 = mybir.dt.int16
    cfg = env["cfg"]
    G = cfg.G
    ni_pad = cfg.ni_pad
    h_dram = env["h_dram"]
    gld = env["gld"]
    staging = env["staging"]
    piwi_t, piwj_t, iota_t = env["piwi_t"], env["piwj_t"], env["iota_t"]
    pibr_t = env.get("pibr_t")
    iibr_t = env.get("iibr_t")
    nz_pib, nz_iib = cfg.nz_pib, cfg.nz_iib

    half = D * NB // 2

    DELAY_EV = 2     # psI evict
    DELAY_EIN = 3    # DVE mult+r1 (frees psI; emitted before compute)
    DELAY_EIN2 = 6   # r2 (DVE)
    DELAY_Z = 8      # z (Pool)
    DELAY_S = 4      # S build
    DELAY_TH = 11    # tanh
    DELAY_SC = 14    # scatter matmul
    K = G * CPG

    with tc.tile_pool(name="eg", bufs=6) as eg, \
         tc.tile_pool(name="gth", bufs=3) as gth, \
         tc.tile_pool(name="ew", bufs=6) as ew, \
         tc.tile_pool(name="late", bufs=DELAY_SC + 6) as late, \
         tc.tile_pool(name="psI", bufs=3, space="PSUM") as psIp, \
         tc.tile_pool(name="psA", bufs=2, space="PSUM") as psAp:
        groups = {}   # g -> (wi_t, wj_t, jr_t, bas_t)
        gaths = {}    # g -> (hTi_g, hTj_g)
        psIs = {}     # k -> psum
        sbIs = {}     # k -> evicted psI
        r1s = {}      # k -> r1
        r2s = {}      # k -> r2
        zs = {}       # k -> z (pre-tanh)
        i1s = {}      # k -> i1
        Ss = {}       # k -> S
        psAs = {}     # g -> psum accumulator

        WIB = GROUP // 16 * 2
        JRB = CPG * 4
        GLB = 2 * WIB + JRB + CPG * NB * 2

        def load_group(g):
            gl_t = eg.tile([128, GLB], mybir.dt.uint8, name="gl", tag="gl")
            nc.sync.dma_start(out=gl_t[:], in_=gld[g])
            wi_t = gl_t[:, 0:WIB].bitcast(i16)
            wj_t = gl_t[:, WIB:2 * WIB].bitcast(i16)
            jr_t = gl_t[:, 2 * WIB:2 * WIB + JRB].bitcast(f32)
            bas_t = gl_t[:, 2 * WIB + JRB:].bitcast(bf16)
            groups[g] = (wi_t, wj_t, jr_t, bas_t)

        GSP = 512  # idxs per dma_gather call (>512 overflows the SWDGE ring)

        def gather_group(g):
            wi_t, wj_t, _, _ = groups[g]
            hTi = gth.tile([D, GROUP], bf16, name="hTi", tag="hTi")
            hTj = gth.tile([D, GROUP], bf16, name="hTj", tag="hTj")
            for s in range(GROUP // GSP):
                c0, c1 = s * (GSP // 16), (s + 1) * (GSP // 16)
                nc.gpsimd.dma_gather(
                    out_ap=hTi[:, s * GSP:(s + 1) * GSP].rearrange(
                        "p (o n) -> p o n", o=1),
                    in_ap=h_dram[0:ni_pad, :],
                    idxs_ap=wi_t[:, c0:c1], num_idxs=GSP, num_idxs_reg=GSP,
                    elem_size=D, transpose=True, queue_num=0)
                nc.gpsimd.dma_gather(
                    out_ap=hTj[:, s * GSP:(s + 1) * GSP].rearrange(
                        "p (o n) -> p o n", o=1),
                    in_ap=h_dram[ni_pad:, :],
                    idxs_ap=wj_t[:, c0:c1], num_idxs=GSP, num_idxs_reg=GSP,
                    elem_size=D, transpose=True, queue_num=1)
            gaths[g] = (hTi, hTj)

        def compute_chunk(k):
            g, q = divmod(k, CPG)
            hTi, hTj = gaths[g]
            sl = slice(q * CHUNK, (q + 1) * CHUNK)
            psI = psIp.tile([CHUNK, D * NB], f32)
            nc.tensor.matmul(out=psI[:, 0:half], lhsT=hTi[:, sl],
                             rhs=piwi_t[:, 0:half], start=True, stop=False)
            nc.tensor.matmul(out=psI[:, half:], lhsT=hTi[:, sl],
                             rhs=piwi_t[:, half:], start=True, stop=False)
            nc.tensor.matmul(out=psI[:, 0:half], lhsT=hTj[:, sl],
                             rhs=piwj_t[:, 0:half], start=False, stop=True)
            nc.tensor.matmul(out=psI[:, half:], lhsT=hTj[:, sl],
                             rhs=piwj_t[:, half:], start=False, stop=True)
            psIs[k] = psI

        # psI eviction split: ACT evicts [0:EVS]; the DVE mult reads the
        # remaining columns straight from PSUM.  With a nonzero pi bias the
        # bias-add must precede the basis mult, so evict everything.
        EVS = D * NB if nz_pib else 896

        def evict_chunk(k):
            psI = psIs[k]
            sbI = ew.tile([CHUNK, EVS], bf16, tag="sbI", bufs=5)
            nc.scalar.activation(sbI[:], psI[:, 0:EVS],
                                 mybir.ActivationFunctionType.Copy)
            if nz_pib:
                nc.vector.tensor_tensor(out=sbI[:], in0=sbI[:],
                                        in1=pibr_t[:, 0:EVS],
                                        op=mybir.AluOpType.add)
            sbIs[k] = sbI

        def einsum_chunk(k):
            g, q = divmod(k, CPG)
            bas_t = groups[g][3]
            sbI = sbIs.pop(k)
            psI = psIs.pop(k)
            prod = ew.tile([CHUNK, D * NB], bf16, tag="prod", bufs=4)
            bb = _bcast_mid(bas_t[:, q * NB:(q + 1) * NB], D)
            nc.vector.tensor_tensor(
                out=prod[:, 0:EVS], in0=sbI[:],
                in1=bass.AP(bb.tensor, bb.offset,
                            [bb.ap[0], [0, EVS // NB], bb.ap[2]]),
                op=mybir.AluOpType.mult)
            if EVS < D * NB:
                tail = bass.AP(bb.tensor, bb.offset,
                               [bb.ap[0], [0, D - EVS // NB], bb.ap[2]])
                nc.vector.tensor_tensor(
                    out=prod[:, EVS:], in0=psI[:, EVS:], in1=tail,
                    op=mybir.AluOpType.mult)
            if k % 2 == 0:
                r1p = ew.tile([CHUNK, D * 8], bf16, tag="r1p", bufs=4)
                r1s[k] = r1p
            else:
                r1p = r1s[k - 1]
            p3 = prod[:].rearrange("p (c b) -> p c b", b=NB)
            half_sl = slice((k % 2) * D * 4, (k % 2 + 1) * D * 4)
            nc.vector.tensor_tensor(
                out=r1p[:, half_sl].rearrange("p (c b) -> p c b", b=4),
                in0=p3[:, :, 0:4], in1=p3[:, :, 4:8],
                op=mybir.AluOpType.add)

        def einsum2_pair(k):
            # k odd: reduce the (k-1, k) pair's r1 (4 -> 2) on DVE
            r1p = r1s.pop(k - 1)
            r2p = ew.tile([CHUNK, D * 4], bf16, tag="r2p", bufs=4)
            r1v = r1p[:].rearrange("p (t c b) -> p t c b", t=2, b=4)
            nc.vector.tensor_tensor(
                out=r2p[:].rearrange("p (t c b) -> p t c b", t=2, b=2),
                in0=r1v[:, :, :, 0:2], in1=r1v[:, :, :, 2:4],
                op=mybir.AluOpType.add)
            r2s[k] = r2p

        def zred_pair(k):
            # k odd: final 2 -> 1 reduction on Pool, one iteration later
            r2p = r2s.pop(k)
            zp = late.tile([CHUNK, D * 2], bf16, tag="zp", bufs=6)
            r2v = r2p[:].rearrange("p (t c b) -> p t c b", t=2, b=2)
            nc.vector.tensor_tensor(
                out=zp[:].rearrange("p (t c) -> p t c", t=2),
                in0=r2v[:, :, :, 0], in1=r2v[:, :, :, 1],
                op=mybir.AluOpType.add)
            zs[k] = zp

        def sbuild_chunk(k):
            g, q = divmod(k, CPG)
            jr_t = groups[g][2]
            S = late.tile([CHUNK, WIN], bf16, tag="S")
            nc.vector.tensor_scalar(
                out=S[:], in0=iota_t[:], scalar1=jr_t[:, q:q + 1],
                scalar2=None, op0=mybir.AluOpType.is_equal)
            Ss[k] = S

        def tanh_pair(k):
            # k odd: tanh over the (k-1, k) pair's z
            zp = zs.pop(k)
            i1p = late.tile([CHUNK, D * 2], bf16, tag="i1p")
            if nz_iib:
                tmp = late.tile([CHUNK, D * 2], bf16, tag="i1tmp")
                nc.vector.tensor_tensor(
                    out=tmp[:].rearrange("p (t c) -> p t c", t=2),
                    in0=zp[:].rearrange("p (t c) -> p t c", t=2),
                    in1=_bcast_mid(iibr_t[:, 0:D], 2),
                    op=mybir.AluOpType.add)
                nc.scalar.activation(i1p[:], tmp[:],
                                     mybir.ActivationFunctionType.Tanh)
            else:
                nc.scalar.activation(i1p[:], zp[:],
                                     mybir.ActivationFunctionType.Tanh)
            i1s[k] = i1p

        def scatter_chunk(k):
            g, q = divmod(k, CPG)
            if q == 0:
                psAs[g] = psAp.tile([D, WIN], f32, name="psA", tag="psA")
            i1p = i1s[k | 1]
            if k % 2 == 1:
                i1s.pop(k | 1)
            sl = slice((k % 2) * D, (k % 2 + 1) * D)
            nc.tensor.matmul(out=psAs[g][:], lhsT=i1p[:, sl],
                             rhs=Ss.pop(k)[:],
                             start=(q == 0), stop=(q == CPG - 1))
            if q == CPG - 1:
                psA = psAs.pop(g)
                acc_sb = eg.tile([D, WIN], f32)
                nc.scalar.activation(acc_sb[:], psA[:],
                                     mybir.ActivationFunctionType.Copy)
                # issue from the ACT queue: acc_sb was just produced there,
                # so the DMA issue never stalls a busy queue head-of-line
                nc.scalar.dma_start(out=staging[g], in_=acc_sb[:])

        GA = 2  # groups of gather lead
        for g in range(min(GA + 1, G)):
            load_group(g)
        for g in range(min(GA, G)):
            gather_group(g)
        for k in range(K + DELAY_SC):
            g, q = divmod(k, CPG)
            if q == 0 and g + GA < G:
                gather_group(g + GA)
            if q == 1 and g + GA + 1 < G:
                load_group(g + GA + 1)
            if 0 <= k - DELAY_EIN < K:
                einsum_chunk(k - DELAY_EIN)
            if k < K:
                compute_chunk(k)
            if 0 <= k - DELAY_EV < K:
                evict_chunk(k - DELAY_EV)
            if 0 <= k - DELAY_S < K:
                sbuild_chunk(k - DELAY_S)
            if 0 <= k - DELAY_EIN2 < K and (k - DELAY_EIN2) % 2 == 1:
                einsum2_pair(k - DELAY_EIN2)
            if 0 <= k - DELAY_Z < K and (k - DELAY_Z) % 2 == 1:
                zred_pair(k - DELAY_Z)
            if 0 <= k - DELAY_TH < K and (k - DELAY_TH) % 2 == 1:
                tanh_pair(k - DELAY_TH)
            if 0 <= k - DELAY_SC < K:
                scatter_chunk(k - DELAY_SC)


# ---------------------------------------------------------------------------
# Entry point
# ---------------------------------------------------------------------------

def _prep_inputs(p1, idx_i, idx_j, basis, pp_w1, pp_b1, pp_w2, pp_b2,
                 pi_w, pi_b, ii_w, ii_b, ncores):
    n_nodes = p1.shape[0]
    cores, G, ni_pad, nj_pad = _plan(
        np.asarray(idx_i), np.asarray(idx_j), np.asarray(basis),
        n_nodes, ncores)
    cfg = _Cfg(G, ni_pad, nj_pad, bool(np.any(pi_b != 0)),
               bool(np.any(ii_b != 0)))

    # fold ii_w into pi_w:  piw_eff[x, (y, b)] = sum_c piw[x, (c, b)] iiw[c, y]
    piw3 = np.asarray(pi_w, np.float64).reshape(2 * D, D, NB)
    piw_eff = np.einsum("xcb,cy->xyb", piw3, np.asarray(ii_w, np.float64))
    piw_eff = piw_eff.reshape(2 * D, D * NB).astype(np.float32)
    pib_eff = np.einsum(
        "cb,cy->yb", np.asarray(pi_b, np.float64).reshape(D, NB),
        np.asarray(ii_w, np.float64)).reshape(D * NB).astype(np.float32)

    common = dict(
        w1=pp_w1.astype(BF16), w2=pp_w2.astype(BF16),
        b1=pp_b1.astype(np.float32).reshape(D, 1),
        b2=pp_b2.astype(np.float32).reshape(D, 1),
        piwi=piw_eff[:D].astype(BF16), piwj=piw_eff[D:].astype(BF16),
    )
    if cfg.nz_pib:
        common["pibr"] = np.tile(pib_eff.astype(BF16)[None, :], (CHUNK, 1))
    if cfg.nz_iib:
        common["iibr"] = np.tile(ii_b.astype(BF16)[None, :], (CHUNK, 1))

    p1f = np.asarray(p1, np.float32)
    in_maps = []
    for c in range(ncores):
        core = cores[c]
        tab = np.zeros((ni_pad + nj_pad, D), np.float32)
        ui = core["uniq_i"]
        tab[:len(ui)] = p1f[ui]
        jmin = core["jmin"]
        jhi = min(jmin + nj_pad, n_nodes)
        tab[ni_pad:ni_pad + (jhi - jmin)] = p1f[jmin:jhi]
        m = dict(common)
        m["tabT"] = np.ascontiguousarray(tab.T).astype(BF16)
        m["gld"] = core["gl"]
        in_maps.append(m)
    return in_maps, cores, cfg, n_nodes


def _assemble(results, cores, n_nodes):
    out = np.zeros((n_nodes, D), np.float32)
    for c, core in enumerate(cores):
        st = results[c]["staging"]
        for g in range(core["ngroups"]):
            base = int(core["bases"][g])
            w = min(WIN, n_nodes - base)
            out[base:base + w] += st[g, :, :w].T
    return out


LAST_RESULTS = None


def kernel(p1, idx_i, idx_j, basis, pp_w1, pp_b1, pp_w2, pp_b2,
           pi_w, pi_b, ii_w, ii_b):
    global LAST_RESULTS
    in_maps, cores, cfg, n_nodes = _prep_inputs(
        p1, idx_i, idx_j, basis, pp_w1, pp_b1, pp_w2, pp_b2,
        pi_w, pi_b, ii_w, ii_b, NCORES)
    nc = _build(cfg)
    res = run_bass_kernel_spmd(nc, in_maps, core_ids=list(range(NCORES)))
    LAST_RESULTS = res
    return _assemble(res.results, cores, n_nodes)
